# revision 12
# baseline (speedup 1.0000x reference)
"""Fused multi-head attention (B=2, N=2048, C=1024, H=16) on 8 TRN2 NeuronCores.

Sharding: core = (b, g) with b = batch (2) and g = head-group of 4 heads (4).
Each core computes, for its batch and 4 heads:
    qkv slice -> per-head softmax attention -> out-proj partial (row-parallel).
Host sums the 4 per-head-group proj partials per batch and adds b_proj.

Device algorithm (per core), matmuls in bf16:
  phase 1: qkT = (x @ Wqk)^T   [q/k feats on partitions, 2048 tokens]
           v   = x @ Wv        [2048 tokens, 4*64]
  phase 2: per (head pair hp, 512-token row chunk rc), 16 key chunks kc --
    all of it built from CONCURRENT tile_position matmul walls (the PE runs
    matmuls on disjoint 32/64-wide array row/col groups simultaneously,
    each with its own moving-operand stream; HW-verified ~3ns stagger):
      wave(kc):  4x 64x64-quadrant MMs = S^T of BOTH heads, 128 keys x 512
                 tokens, in ~512 array cycles (K=64 contraction would
                 otherwise waste half the array).
      exp:       ONE [128,1024] ScalarE ACT covers both heads' tiles.
      PV(kc):    2 col-tiled MMs (M=64 each): h0 -> psum partitions 0:64,
                 h1 -> 64:128, ONE bank, ~512 cycles for both.
      den wall:  per 2 kc, 4 concurrent M=1 MMs (ones stationary) against
                 the 4 ex slices -> softmax denominators accumulate on
                 psum partitions 0/32/64/96 of one bank (~512 cycles per
                 2048 keys*... tokens -- the old ones-column trick cost a
                 second full M=65 wall per head).
  phase 3: partial = out^T-matmul Wp -> bf16 -> DMA out

Schedule: ONE global software pipeline over 64 super-steps (2 kc each).
Per super-step the PE program is
  [wave(2k), wave(2k+1)] [fill pops] [PV walls + den wall, lag-6]
with the two ScalarE ACTs between.  Wave pairs and PV walls run
back-to-back to hide the PE pipe-drain + LDWEIGHTS at config switches.
The lag-6 PV emission (PV(s) depends on ACT(s)) plus fills-before-PVs
keeps the in-order PE queue from head-of-line blocking on ACT completion.
PE is the overall bottleneck (~145us of matmul streams incl. the qkv/proj
fill work vs ~131us ScalarE), so the goal is a never-idle PE; proj(rc=2)
pops are saved for the tail block where the PE otherwise drains early.
The prologue is minimal: qk00+qk20 then v0+v1, each pair interleaved
chunk-by-chunk so both psums chase the same DMA stream.
Note: the box drifts between "fast" and ~20% slower power states on minute
timescales; compare variants only via interleaved runs in one process.
"""

import os
from contextlib import ExitStack

import numpy as np

import concourse.bass as bass
import concourse.mybir as mybir
import concourse.tile as tile
from concourse import bacc
from concourse.bass_utils import run_bass_kernel_spmd

B, N, C = 2, 2048, 1024
HC = 4  # heads per core
D = 64
NCORES = 8
KC = C // 128  # 8 contraction chunks for phase 1
SCALE = D**-0.5  # 0.125
LAG = int(os.environ.get("ATTN_LAG", "6"))  # PV emission lag (even, >=2)

# "f32r" (fp32 data, full-rate PE mode), "bf16", or "f32" (4x slower PE)
MM_DT = os.environ.get("ATTN_MM_DT", "bf16")


def _np_in_dtype():
    if MM_DT == "bf16":
        import ml_dtypes

        return np.dtype(ml_dtypes.bfloat16)
    return np.dtype(np.float32)


def _prep(a):
    """Cast to the device input dtype; for f32r, pre-round to TF32 (RTNE)."""
    a = np.ascontiguousarray(a)
    if MM_DT != "f32r":
        return a.astype(_np_in_dtype())
    u = a.astype(np.float32).view(np.uint32)
    u = (u + 0x0FFF + ((u >> 13) & 1)) & np.uint32(0xFFFFE000)
    return u.view(np.float32)


def build_nc():
    f32 = mybir.dt.float32
    in_dt = {
        "bf16": mybir.dt.bfloat16,
        "f32r": mybir.dt.float32r,
        "f32": mybir.dt.float32,
    }[MM_DT]
    mm = lambda ap: ap  # noqa: E731

    out_dt = mybir.dt.bfloat16 if MM_DT == "bf16" else f32

    nc = bacc.Bacc("TRN2", target_bir_lowering=False, debug=False, num_devices=NCORES)
    xT_d = nc.dram_tensor("xT", [C, N], in_dt, kind="ExternalInput").ap()
    wqk_d = nc.dram_tensor("wqk", [C, 2 * HC * D], in_dt, kind="ExternalInput").ap()
    wv_d = nc.dram_tensor("wv", [C, HC * D], in_dt, kind="ExternalInput").ap()
    wp_d = nc.dram_tensor("wp", [HC * D, C], in_dt, kind="ExternalInput").ap()
    # bf16 proj partials: halves the output DMA (the tail's critical path);
    # the host accumulates the 4 partials per batch in f32.
    out_d = nc.dram_tensor("out", [N, C], out_dt, kind="ExternalOutput").ap()

    with tile.TileContext(nc) as tc:
        with (
            tc.tile_pool(name="const", bufs=1) as const,
            tc.tile_pool(name="ex", bufs=8) as expool,
            tc.tile_pool(name="den", bufs=6) as dpool,
            tc.tile_pool(name="stage", bufs=4) as stage,
            tc.tile_pool(name="stps", bufs=2, space="PSUM") as stps,
            tc.tile_pool(name="pvps", bufs=4, space="PSUM") as pvps,
        ):
            # persistent tiles
            # qkT chunks: 0 = q heads 0,1; 1 = q heads 2,3
            #   (head even -> partitions 0:64, odd -> 64:128)
            # kT2: same layout for k -- consumed in 64x64 slices by the
            #   quadrant ST matmuls, so no zero padding is needed.
            qkT_sb = const.tile([128, 2, N], in_dt, tag="qkT")
            kT2_sb = const.tile([128, 2, N], in_dt, tag="kT2")
            v_sb = const.tile([128, 16, HC, D], in_dt, tag="v")
            wp_sb = const.tile([128, 2, C], in_dt, tag="wp")
            outT_sb = const.tile([128, 2, N], in_dt, tag="outT")
            xT_sb = const.tile([128, KC, N], in_dt, tag="xT")
            wqk_sb = const.tile([128, KC, 2 * HC * D], in_dt, tag="wqk")
            wv_sb = const.tile([128, KC, HC * D], in_dt, tag="wv")

            # ---- DMAs: the prologue-critical wqk / xT(nt=0) / wv chunks go
            # one-per-queue (Sync / GpSimd / Scalar HW DMA queues) so all
            # three stream in parallel and the interleaved qk00/qk20 (and
            # v0/v1) prologue contractions can follow the data chunk-by-chunk.
            for kc in range(KC):
                nc.sync.dma_start(
                    wqk_sb[:, kc, :], wqk_d[kc * 128 : (kc + 1) * 128, :]
                )
                nc.gpsimd.dma_start(
                    xT_sb[:, kc, 0:512], xT_d[kc * 128 : (kc + 1) * 128, 0:512]
                )
                nc.scalar.dma_start(
                    wv_sb[:, kc, :], wv_d[kc * 128 : (kc + 1) * 128, :]
                )

            def dma_xt(nt, q):
                for kc in range(KC):
                    q.dma_start(
                        xT_sb[:, kc, nt * 512 : (nt + 1) * 512],
                        xT_d[kc * 128 : (kc + 1) * 128, nt * 512 : (nt + 1) * 512],
                    )

            dma_xt(1, nc.sync)
            dma_xt(2, nc.gpsimd)
            dma_xt(3, nc.sync)
            for c2 in range(2):
                nc.scalar.dma_start(wp_sb[:, c2, :], wp_d[c2 * 128 : (c2 + 1) * 128, :])

            # ---- one-time fills (run during the DMA wait) ----
            zbf = const.tile([64, 512], in_dt, tag="zbf")
            nc.vector.memset(zbf[:], 0.0)
            zsrc = const.tile([64, 512], f32, tag="zsrc")
            nc.vector.memset(zsrc[:], 0.0)
            onesb = const.tile([128, 1], in_dt, tag="onesb")
            nc.vector.memset(onesb[:], 1.0)

            # dependency-free bf16 fillers (never consumed): a couple up
            # front tickle the PE HAM activity window during the DMA wait.
            wps = stps.tile([128, 1024], f32, tag="st", name="wps")

            def filler(n=256):
                nc.tensor.matmul(
                    wps[:, 0:n], zbf[:, 0:128], zbf[:, 0:n], start=True, stop=True
                )

            # ---- emission helpers ----
            def qk_chunk(mf, nt):
                """One psum of (x @ Wqk)^T: feat chunk mf, token chunk nt.
                wqk feat chunks: 0 = q heads 0,1; 1 = q heads 2,3;
                2 = k heads 0,1; 3 = k heads 2,3.
                NOTE: the psum accumulation must be fully emitted in one pop
                -- leaving it open across pops lets the pool hand its slot
                to another fill, whose start=True clears the live bank."""
                ps = pvps.tile([128, 512], f32, tag="pv", name="pv")
                for kc in range(KC):
                    nc.tensor.matmul(
                        ps,
                        mm(wqk_sb[:, kc, mf * 128 : (mf + 1) * 128]),
                        mm(xT_sb[:, kc, nt * 512 : (nt + 1) * 512]),
                        start=(kc == 0),
                        stop=(kc == KC - 1),
                    )
                nts = slice(nt * 512, (nt + 1) * 512)
                if mf < 2:
                    nc.vector.tensor_copy(qkT_sb[:, mf, nts], ps)
                else:
                    nc.vector.tensor_copy(kT2_sb[:, mf - 2, nts], ps)

            def v_chunk(t):
                """One psum of v = x @ Wv for token(=key) chunk t, all heads."""
                ps = pvps.tile([128, 512], f32, tag="pv", name="pv")[:, : HC * D]
                for kc in range(KC):
                    nc.tensor.matmul(
                        ps,
                        mm(xT_sb[:, kc, t * 128 : (t + 1) * 128]),
                        mm(wv_sb[:, kc, :]),
                        start=(kc == 0),
                        stop=(kc == KC - 1),
                    )
                nc.vector.tensor_copy(
                    v_sb[:, t, :, :], ps.rearrange("p (h d) -> p h d", h=HC)
                )

            sg2_of = {}

            def proj_chunk(t, nf):
                """partial[t*128:(t+1)*128, nf*512:(nf+1)*512] = out @ Wp.
                Both nf halves stage into one [128,1024] tile; the DMA (2KB
                rows, half the packets) fires once per token chunk."""
                ps = pvps.tile([128, 512], f32, tag="pv", name="pv")
                for c2 in range(2):
                    nc.tensor.matmul(
                        ps,
                        mm(outT_sb[:, c2, t * 128 : (t + 1) * 128]),
                        mm(wp_sb[:, c2, nf * 512 : (nf + 1) * 512]),
                        start=(c2 == 0),
                        stop=(c2 == 1),
                    )
                if nf == 0:
                    sg2_of[t] = stage.tile(
                        [128, 1024], out_dt, tag="sg2", name="sg2", bufs=2
                    )
                sg = sg2_of[t]
                nc.vector.tensor_copy(sg[:, nf * 512 : (nf + 1) * 512], ps)
                if nf == 1:
                    nc.sync.dma_start(out_d[t * 128 : (t + 1) * 128, :], sg)
                    del sg2_of[t]

            def proj_tail(t):
                """Both nf halves of token chunk t in one stps-pool psum
                (free after the last exp): fewer, wider tail ops + 2KB-row
                output DMA."""
                ps = stps.tile([128, 1024], f32, tag="st", name="st")
                for nf in range(2):
                    for c2 in range(2):
                        nc.tensor.matmul(
                            ps[:, nf * 512 : (nf + 1) * 512],
                            mm(outT_sb[:, c2, t * 128 : (t + 1) * 128]),
                            mm(wp_sb[:, c2, nf * 512 : (nf + 1) * 512]),
                            start=(c2 == 0),
                            stop=(c2 == 1),
                        )
                sg = stage.tile([128, 1024], out_dt, tag="sg2", name="sg2", bufs=2)
                nc.vector.tensor_copy(sg, ps)
                # tail runs after the last ACT, so the scalar queue is free
                (nc.sync if t % 2 == 0 else nc.scalar).dma_start(
                    out_d[t * 128 : (t + 1) * 128, :], sg
                )

            # fill queue: work interleaved into the pipeline's PE slack
            fills = []

            def queue_proj(rc):
                fills.extend(
                    [
                        lambda t=t, nf=nf: proj_chunk(t, nf)
                        for t in range(4 * rc, 4 * rc + 4)
                        for nf in range(2)
                    ]
                )

            # ---- prologue: exactly what super-step 0 needs, contracted
            # chunk-by-chunk as the DMA delivers.  qk00/qk20 (then v0/v1)
            # share input chunks, so they interleave kc-wise and each DMA
            # arrival feeds two back-to-back matmuls.
            filler()
            filler()
            psA = pvps.tile([128, 512], f32, tag="pv", name="psA")
            psB = pvps.tile([128, 512], f32, tag="pv", name="psB")
            for kc in range(KC):
                for ps, mf in ((psA, 0), (psB, 2)):
                    nc.tensor.matmul(
                        ps,
                        mm(wqk_sb[:, kc, mf * 128 : (mf + 1) * 128]),
                        mm(xT_sb[:, kc, 0:512]),
                        start=(kc == 0),
                        stop=(kc == KC - 1),
                    )
            nc.vector.tensor_copy(qkT_sb[:, 0, 0:512], psA)
            nc.vector.tensor_copy(kT2_sb[:, 0, 0:512], psB)
            psC = pvps.tile([128, 512], f32, tag="pv", name="psC")
            psD = pvps.tile([128, 512], f32, tag="pv", name="psD")
            for kc in range(KC):
                for ps, t in ((psC, 0), (psD, 1)):
                    nc.tensor.matmul(
                        ps[:, : HC * D],
                        mm(xT_sb[:, kc, t * 128 : (t + 1) * 128]),
                        mm(wv_sb[:, kc, :]),
                        start=(kc == 0),
                        stop=(kc == KC - 1),
                    )
            for ps, t in ((psC, 0), (psD, 1)):
                nc.vector.tensor_copy(
                    v_sb[:, t, :, :],
                    ps[:, : HC * D].rearrange("p (h d) -> p h d", h=HC),
                )

            # ---- attention: ONE global software pipeline over 64 supers ----
            border = [
                (0, 0),
                (0, 1),
                (1, 0),
                (0, 2),
                (1, 1),
                (0, 3),
                (1, 2),
                (1, 3),
            ]
            # fill order + per-super pop counts: qk chunks split into halves
            # (~0.9us each) so no single pop swamps a super-step; v(kc) and
            # kT2/q chunks land >=1 super before their consumer (waves are
            # lag-0, PVs lag-6); proj(0..1) spread mid-run; proj(2) saved
            # for the ACT-backlog tail where the PE otherwise drains early.
            fills.extend(
                [
                    lambda: qk_chunk(2, 1),
                    lambda: v_chunk(2),
                    lambda: v_chunk(3),
                    lambda: qk_chunk(2, 2),
                    lambda: v_chunk(4),
                    lambda: v_chunk(5),
                    lambda: v_chunk(6),
                    lambda: qk_chunk(2, 3),
                    lambda: v_chunk(7),
                    lambda: v_chunk(8),
                    lambda: v_chunk(9),
                    lambda: qk_chunk(0, 1),
                    lambda: v_chunk(10),
                    lambda: v_chunk(11),
                    lambda: v_chunk(12),
                    lambda: v_chunk(13),
                ]
            )
            fills2 = [
                lambda: v_chunk(14),
                lambda: v_chunk(15),
                lambda: qk_chunk(1, 0),
                lambda: qk_chunk(3, 0),
                lambda: qk_chunk(3, 1),
                lambda: qk_chunk(3, 2),
                lambda: qk_chunk(3, 3),
                lambda: qk_chunk(0, 2),
                lambda: qk_chunk(1, 1),
            ]
            fills3 = [
                lambda: qk_chunk(0, 3),
                lambda: qk_chunk(1, 2),
                lambda: qk_chunk(1, 3),
            ]
            sched = (
                [1, 2, 2, 2, 2, 2, 3, 2]  # block0: kT2 chunks + qk01 + v2-13
                + [2, 1, 1, 1, 1, 1, 1, 1]  # block1: v14,v15 + pair-1 q/kT2
                + [1, 1, 1, 0, 0, 0, 0, 0]  # block2: fills3 (q chunks)
                + [0, 0, 0, 1, 1, 1, 1, 1]  # block3: proj0 (queued ~s26)
                + [1, 1, 1, 0, 0, 0, 0, 0]  # block4
                + [0, 0, 0, 1, 1, 1, 1, 1]  # block5: proj1 (queued ~s42)
                + [1, 1, 1, 0, 0, 0, 0, 0]  # block6
                + [0, 0, 1, 1, 1, 1, 2, 2]  # block7: proj2 -> PE-starved tail
            )
            bst = [None] * 8  # per-block pipeline state
            exs = {}

            def wave(s):
                bi, kc = s // 16, s % 16
                if kc == 0:
                    hp, rc = border[bi]
                    rcs = slice(rc * 512, (rc + 1) * 512)
                    bst[bi] = {
                        "heads": (2 * hp, 2 * hp + 1),
                        "pv": pvps.tile([128, 512], f32, tag="pv", name="pvb"),
                        "den": pvps.tile([128, 512], f32, tag="pv", name="denb"),
                        "q": (qkT_sb[0:64, hp, rcs], qkT_sb[64:128, hp, rcs]),
                    }
                    if bi == 1:
                        fills.extend(fills2)
                    if bi == 2:
                        fills.extend(fills3)
                hp, rc = border[bi]
                st8 = bst[bi]
                st = stps.tile([128, 1024], f32, tag="st", name="st")
                kb = kc * 128
                for hh in range(2):  # head within pair -> array row half
                    for cc in range(2):  # key half -> array col half
                        nc.tensor.matmul(
                            st[64 * cc : 64 * cc + 64, 512 * hh : 512 * hh + 512],
                            mm(
                                kT2_sb[
                                    64 * hh : 64 * hh + 64,
                                    hp,
                                    kb + 64 * cc : kb + 64 * cc + 64,
                                ]
                            ),
                            mm(st8["q"][hh]),
                            start=True,
                            stop=True,
                            tile_position=(64 * hh, 64 * cc),
                        )
                return st

            def act(s, st):
                ex = expool.tile([128, 1024], in_dt, tag="ex", name="ex")
                nc.scalar.activation(
                    ex, st, mybir.ActivationFunctionType.Exp, scale=SCALE
                )
                exs[s] = ex

            def pv_walls(s0):
                """PV walls for kc pair (s0, s0+1) + the shared den wall.
                Each PV wall: both heads concurrent (col-tiled M=64, h0 ->
                pvb[0:64], h1 -> pvb[64:128]).  Den wall: 4 concurrent M=1
                MMs accumulate denominators at denb partitions 0/32/64/96
                (h0/h1 x even/odd kc)."""
                bi = s0 // 16
                st8 = bst[bi]
                pvb, denb = st8["pv"], st8["den"]
                exab = (exs.pop(s0), exs.pop(s0 + 1))
                for j, ex in enumerate(exab):
                    kc = (s0 + j) % 16
                    for hh, h in enumerate(st8["heads"]):
                        nc.tensor.matmul(
                            pvb[64 * hh : 64 * hh + 64, :],
                            mm(v_sb[:, kc, h, :]),
                            mm(ex[:, 512 * hh : 512 * hh + 512]),
                            start=(kc == 0),
                            stop=(kc == 15),
                            tile_position=(0, 64 * hh),
                        )
                for j, ex in enumerate(exab):
                    kc = (s0 + j) % 16
                    for hh in range(2):
                        p = 64 * j + 32 * hh
                        nc.tensor.matmul(
                            denb[p : p + 1, :],
                            mm(onesb[:]),
                            mm(ex[:, 512 * hh : 512 * hh + 512]),
                            start=(kc < 2),
                            stop=(kc >= 14),
                            tile_position=(0, p),
                        )
                if (s0 + 1) % 16 == 15:
                    block_end(bi)

            def block_end(bi):
                """Denominator chain + normalize (+ tail proj) for block bi."""
                hp, rc = border[bi]
                st8 = bst[bi]
                heads = st8["heads"]
                pvb, denb = st8["pv"], st8["den"]
                tail = bi == 7
                if tail:
                    # the tail denominator chain leaves the PE idle just over
                    # the HAM re-throttle window; dependency-free filler
                    # matmuls (f32 on zsrc, never consumed) bridge it.
                    wmt = stps.tile([128, 1024], f32, tag="st", name="wmt")
                    for _ in range(8):
                        nc.tensor.matmul(
                            wmt[:, 0:256],
                            zsrc[:, 0:128],
                            zsrc[:, 0:256],
                            start=True,
                            stop=True,
                        )
                # den_h = even-kc row + odd-kc row, then approx-fast recip
                # (~51 ULP -- the denominator only needs ~1e-2 relative).
                # both adds/recips first: DVE stays busy while GpSimd runs
                # the first broadcast.
                dens, rbcs = {}, {}
                for hh, h in enumerate(heads):
                    # DVE may read only ONE input from PSUM: stage the
                    # even-kc row to SBUF, then add the odd-kc PSUM row.
                    dcp = dpool.tile([1, 512], f32, tag="dcp", name="dcp")
                    nc.vector.tensor_copy(dcp, denb[32 * hh : 32 * hh + 1, :])
                    dsum = dpool.tile([1, 512], f32, tag="dsrc", name="dsrc")
                    nc.vector.tensor_tensor(
                        out=dsum,
                        in0=dcp,
                        in1=denb[64 + 32 * hh : 64 + 32 * hh + 1, :],
                        op=mybir.AluOpType.add,
                    )
                    dens[h] = dpool.tile([1, 512], f32, tag="den", name="den")
                    nc.vector.reciprocal_approx_fast(out=dens[h], in_=dsum)
                for h in heads:
                    rbcs[h] = dpool.tile([64, 512], f32, tag="rbc", name="rbc")
                    nc.gpsimd.partition_broadcast(rbcs[h], dens[h])
                if not tail:
                    for hh, h in enumerate(heads):
                        nc.vector.tensor_tensor(
                            out=outT_sb[
                                64 * hh : 64 * hh + 64, hp, rc * 512 : (rc + 1) * 512
                            ],
                            in0=pvb[64 * hh : 64 * hh + 64, :],
                            in1=rbcs[h][:],
                            op=mybir.AluOpType.mult,
                        )
                else:
                    # final block: normalize per 128-token chunk and launch
                    # that chunk's out-proj + DMA immediately, so the tail
                    # pipeline (mult -> proj MM -> cast -> DMA) overlaps
                    # instead of serializing after the whole block.
                    for tc4 in range(4):
                        ts = slice(rc * 512 + tc4 * 128, rc * 512 + tc4 * 128 + 128)
                        for hh, h in enumerate(heads):
                            nc.vector.tensor_tensor(
                                out=outT_sb[64 * hh : 64 * hh + 64, hp, ts],
                                in0=pvb[
                                    64 * hh : 64 * hh + 64,
                                    tc4 * 128 : (tc4 + 1) * 128,
                                ],
                                in1=rbcs[h][:, tc4 * 128 : (tc4 + 1) * 128],
                                op=mybir.AluOpType.mult,
                            )
                        proj_tail(4 * rc + tc4)
                if hp == 1 and rc < 3:
                    queue_proj(rc)

            for sup in range(64):
                s0, s1 = 2 * sup, 2 * sup + 1
                stA = wave(s0)
                stB = wave(s1)
                act(s0, stA)
                act(s1, stB)
                for _ in range(sched[sup]):
                    if fills:
                        fills.pop(0)()
                if s0 >= LAG:
                    pv_walls(s0 - LAG)
            for s0 in range(128 - LAG, 128, 2):
                pv_walls(s0)
            # drain any straggler fills (none expected)
            while fills:
                fills.pop(0)()
    nc.compile()
    return nc


def make_in_maps(x, w_qkv, w_proj):
    in_maps = []
    for core in range(NCORES):
        b, g = core // 4, core % 4
        qs = slice(g * 256, (g + 1) * 256)
        in_maps.append(
            {
                "xT": _prep(x[b].T),
                "wqk": _prep(
                    np.concatenate(
                        [w_qkv[:, qs], w_qkv[:, C + g * 256 : C + (g + 1) * 256]],
                        axis=1,
                    )
                ),
                "wv": _prep(w_qkv[:, 2 * C + g * 256 : 2 * C + (g + 1) * 256]),
                "wp": _prep(w_proj[qs, :]),
            }
        )
    return in_maps


def run_hw(x, w_qkv, w_proj, b_proj, trace=False, tmpdir=None):
    """Returns (full output [2, 2048, 1024] f32, exec_time_ns or None)."""
    in_maps = make_in_maps(x, w_qkv, w_proj)
    nc = build_nc()
    r = run_bass_kernel_spmd(
        nc, in_maps, core_ids=list(range(NCORES)), trace=trace, tmpdir=tmpdir
    )
    full = np.zeros((B, N, C), np.float32)
    for core in range(NCORES):
        full[core // 4] += np.asarray(r.results[core]["out"], dtype=np.float32)
    full += np.asarray(b_proj, np.float32)[None, None, :]
    return full, r.exec_time_ns


def kernel(**inputs):
    x = np.asarray(inputs["x"], np.float32)
    w_qkv = np.asarray(inputs["w_qkv"], np.float32)
    w_proj = np.asarray(inputs["w_proj"], np.float32)
    b_proj = np.asarray(inputs["b_proj"], np.float32)
    out, _ = run_hw(x, w_qkv, w_proj, b_proj, trace=False)
    return out
